# revision 11
# baseline (speedup 1.0000x reference)
"""Trainium2 Bass kernel for nn_CategoryAdder (embedding lookup + masked add).

Computation: out[b,s,:] = inputs[b,s,:] + emb where
  emb = table[categories[b,s]] masked to zero when categories[b,s]==0 or
  s == mask_positions[b].

Host-side preprocessing folds both masks into the data:
  - categories[b, mask_positions[b]] = 0
  - table row 0 zeroed (on a copy)
so the device computes exactly: out = inputs + table0[categories].

Numeric scheme: x and the table are both int8, quantized host-side with ONE
shared scale S = max(absmax(x), absmax(table))/127. The device adds raw int8
codes on the DVE with int8 output — measured on TRN2, the DVE saturates the
int8 writeback to [-128,127], so the rare |qx+qt|>127 sums clip instead of
wrapping (measured rel err 1.28e-2 vs the 2e-2 gate; clipping contributes
~4e-3). The host multiplies by S during the free fp32 conversion. Byte
footprint per core: 8.4 (x) + 8.4 (gather) + 8.4 (out) MB = 25.2MB, a 70us
DMA-bus roofline (16 engines x 22.5 GB/s).

Bottleneck structure (from NTFF profiles): the SWDGE gather's Q7 descriptor
generation runs ~1.9ns/desc uncontended but 6-9ns/desc while the DVE runs
adds (Pool shares an SBUF port with the DVE), and a 2048-desc gather
overflows the default 1024-desc/queue ring so gen stalls mid-instruction on
drain. Mitigations here:
  - Every gather tile is resident in SBUF (qp pool = one buf per tile), so
    gather dispatch never waits on the DVE add chain to recycle a slot.
  - A 16-index warmup gather pays the ~6us first-call Q7 IRAM load while
    the idx DMAs are still in flight.
  - inp bufs=6 so x loads enter the HWDGE ring well ahead of the stores
    (a late x load starved the DVE for 10us in the v5 trace).
  - 4 SWDGE queues rotate so drains overlap gen of the next gather.
  - Tile schedule is small at both ends: small head tiles prime the pipe
    (first add by ~8us), small tail tiles shrink the serial
    last-gather -> last-add -> last-store chain.
  - The last tiles' x is preloaded up front so final adds never queue
    behind stores in the HWDGE ring FIFO; adds/stores are pinned to tile
    order with ordering-only deps (the scheduler's cost model
    underestimates gather gen ~20x and otherwise hoists late-tile adds,
    serializing the tail).

Sharding: data-parallel over batch across 8 NeuronCores (8 batches per core,
16384 tokens/core). Table replicated. Per tile of T tokens: SWDGE dma_gather
pulls 512B int8 table rows from HBM by precomputed int16 indices, HWDGE loads
the int8 input tile, DVE adds the codes (saturating int8), HWDGE stores int8.
"""

import numpy as np
import ml_dtypes

import concourse.mybir as mybir
from concourse import bacc, tile
from concourse.bass_utils import run_bass_kernel_spmd
from concourse.tile import add_dep_helper

BF16 = ml_dtypes.bfloat16


def _ensure_axon_ntff_hook_module():
    """run_bass_kernel_spmd(trace=True) under axon imports antenv.axon_hooks,
    which this image lacks — install a fallback shim (backed by the boot
    module's ctypes hook when available) so a BASS_TRACE=1 environment does
    not crash the kernel. No-op when the real module exists."""
    try:
        import antenv.axon_hooks  # noqa: F401
        return
    except ImportError:
        pass
    import sys
    import types

    hook = None
    try:
        import trn_agent_boot.trn_boot as _tb

        hook = _tb._ntff_profile_via_ctypes("/opt/axon/libaxon_pjrt.so")
    except Exception:
        hook = None  # get_..._hook() -> None makes bass_utils skip tracing
    mod = types.ModuleType("antenv.axon_hooks")
    mod.get_axon_ntff_profile_hook = lambda: hook
    mod.set_axon_ntff_profile_hook = lambda h: None
    sys.modules["antenv.axon_hooks"] = mod


_ensure_axon_ntff_hook_module()

B, S, D = 64, 2048, 512
N_CAT = 5000
N_CORES = 8
B_PER = B // N_CORES          # 8 batches per core
NTOK = B_PER * S              # 16384 tokens per core
IDX_COLS = NTOK // 16         # columns of the wrapped int16 index tensor

# Tile schedule (tokens per tile): small tiles prime the pipeline at the start
# (first gather gen is ~0.5us, so the first add can start by ~8us) and small
# tail tiles shorten the serial gather->add->store chain at the end.
TILES = [256, 256, 512, 1024] + [2048] * 6 + [1024, 512, 256, 256]
assert sum(TILES) == NTOK
N_HEAD = 3  # tiles whose indices ride the small head idx DMA
N_TAIL = 3  # tiles whose x is preloaded at start (dedicated pool) so the
            # last adds never wait on x-loads queued behind big stores


def _build_nc():
    nc = bacc.Bacc(
        "TRN2",
        target_bir_lowering=False,
        debug=False,
        num_swdge_queues=4,
    )
    x = nc.dram_tensor("x", [NTOK, D], mybir.dt.int8, kind="ExternalInput")
    tbl = nc.dram_tensor("tbl", [N_CAT, D], mybir.dt.int8, kind="ExternalInput")
    idx = nc.dram_tensor("idx", [128, IDX_COLS], mybir.dt.int16, kind="ExternalInput")
    out = nc.dram_tensor("out", [NTOK, D], mybir.dt.int8, kind="ExternalOutput")

    head = sum(t // 16 for t in TILES[:N_HEAD])
    with tile.TileContext(nc) as tc:
        with (
            tc.tile_pool(name="idxp", bufs=1) as idxp,
            tc.tile_pool(name="inp", bufs=6) as inp,
            # One resident gather tile per schedule tile: gathers never wait
            # on the DVE add chain to free a slot (v3 trace showed 8.5us
            # per-gather waits on qp reuse), so all SWDGE gen+drain finishes
            # early and the add chain runs gather-unblocked.
            tc.tile_pool(name="qp", bufs=len(TILES)) as qp,
            tc.tile_pool(name="outp", bufs=4) as outp,
            tc.tile_pool(name="tailp", bufs=N_TAIL) as tailp,
        ):
            # Warmup: a 16-index gather of row 0 issued before anything
            # else pays the ~6us first-call Q7 IRAM load and cold-dispatch
            # cost while the idx DMAs are still in flight, so the first real
            # gather starts ~5us earlier.
            warm_idx = idxp.tile([128, 1], mybir.dt.int16, tag="warmi")
            nc.vector.memset(warm_idx[:], 0)
            warm_q = idxp.tile([128, 512], mybir.dt.int8, tag="warmq")
            nc.gpsimd.dma_gather(
                warm_q[:].rearrange("p (c e) -> p c e", e=D),
                tbl[:, :],
                warm_idx[:, :],
                16,
                16,
                D,
                single_packet=False,
                queue_num=0,
            )
            # Two separate idx tiles (separate semaphores): the first gather
            # only waits on the small head DMA, not the full idx transfer.
            idx_head = idxp.tile([128, head], mybir.dt.int16, tag="idxh")
            idx_tail = idxp.tile([128, IDX_COLS - head], mybir.dt.int16, tag="idxt")
            nc.sync.dma_start(out=idx_head[:], in_=idx[:, :head])
            nc.sync.dma_start(out=idx_tail[:], in_=idx[:, head:])
            # Preload the tail tiles' x up front: issued now, these loads sit
            # ahead of all stores in the HWDGE ring FIFO, so the final adds
            # are never stuck behind store drains.
            tail_x = []
            t0 = NTOK - sum(TILES[-N_TAIL:])
            for T in TILES[-N_TAIL:]:
                xt = tailp.tile([128, (T // 128) * D], mybir.dt.int8, tag="tx")
                nc.sync.dma_start(
                    out=xt[:],
                    in_=x[t0 : t0 + T].rearrange("(p c) e -> p (c e)", p=128),
                )
                tail_x.append(xt)
                t0 += T
            t0 = 0
            col = 0
            prev_add = None
            prev_store = None
            for ti, T in enumerate(TILES):
                C = T // 128
                if ti < N_HEAD:
                    idx_ap = idx_head[:, col : col + T // 16]
                else:
                    idx_ap = idx_tail[:, col - head : col - head + T // 16]
                q_t = qp.tile([128, C * D], mybir.dt.int8, tag="q")
                nc.gpsimd.dma_gather(
                    q_t[:].rearrange("p (c e) -> p c e", e=D),
                    tbl[:, :],
                    idx_ap,
                    T,
                    T,
                    D,
                    # multi-packet lets the SDMA engines start draining while
                    # Q7 is still generating descriptors; single_packet also
                    # hard-fails above 1024 idxs.
                    single_packet=False,
                    # Alternate SWDGE queues: separate descriptor rings let
                    # drains overlap the next gather's generation.
                    queue_num=ti % 4,
                )
                if ti >= len(TILES) - N_TAIL:
                    in_t = tail_x[ti - (len(TILES) - N_TAIL)]
                else:
                    in_t = inp.tile([128, C * D], mybir.dt.int8, tag="in")
                    nc.sync.dma_start(
                        out=in_t[:],
                        in_=x[t0 : t0 + T].rearrange("(p c) e -> p (c e)", p=128),
                    )
                # Saturating integer-code add on the DVE: int8+int8 -> int8
                # clamps to [-128,127] on writeback (hardware-verified).
                o_t = outp.tile([128, C * D], mybir.dt.int8, tag="o")
                add_i = nc.vector.tensor_add(out=o_t[:], in0=q_t[:], in1=in_t[:])
                store_i = nc.sync.dma_start(
                    out=out[t0 : t0 + T].rearrange("(p c) e -> p (c e)", p=128),
                    in_=o_t[:],
                )
                # Pin adds/stores to tile order with ordering-only edges:
                # the scheduler's cost model thinks gather gen is ~free and
                # otherwise hoists late-tile adds ahead of mid-tile ones,
                # serializing the tail behind the slowest gathers.
                if prev_add is not None:
                    add_dep_helper(
                        add_i.ins, prev_add.ins, sync=False, reason="pin add order"
                    )
                    add_dep_helper(
                        store_i.ins, prev_store.ins, sync=False,
                        reason="pin store order",
                    )
                prev_add, prev_store = add_i, store_i
                t0 += T
                col += T // 16
    nc.compile()
    return nc


def _prep_idx(cat_shard: np.ndarray) -> np.ndarray:
    """cat_shard: (NTOK,) int -> wrapped int16 index tensor [128, IDX_COLS].

    dma_gather writes gather-slot i to SBUF (partition i%128, column i//128);
    our tiles place token t at (partition t//C, column t%C), so slot i holds
    the category of token (i%128)*C + i//128. Indices are then wrapped 16-way
    (idxs[p, s] = slot s*16+p) and replicated across the 8 groups of 16
    partitions as the HW expects.
    """
    blocks = []
    t0 = 0
    for T in TILES:
        C = T // 128
        slot_to_token = (np.arange(T) % 128) * C + (np.arange(T) // 128)
        vals = cat_shard[t0 : t0 + T][slot_to_token]
        blocks.append(np.tile(vals.reshape(T // 16, 16).T, (8, 1)))
        t0 += T
    return np.ascontiguousarray(np.concatenate(blocks, axis=1).astype(np.int16))


RUN_KWARGS = {}  # test harness can set e.g. {"trace": True}
LAST_RESULTS = None
_NC = None


def _get_nc():
    global _NC
    if _NC is None:
        _NC = _build_nc()
    return _NC


def kernel(inputs, categories, mask_positions, table):
    global LAST_RESULTS
    inputs = np.asarray(inputs, dtype=np.float32)
    categories = np.asarray(categories).astype(np.int64)
    mask_positions = np.asarray(mask_positions).astype(np.int64)
    table = np.asarray(table, dtype=np.float32)

    # Fold both masks into the data.
    cat = categories.copy()
    cat[np.arange(B), mask_positions[:, 0]] = 0
    tbl0 = table.astype(np.float32)
    tbl0[0] = 0.0
    sg = np.float32(max(np.abs(tbl0).max(), np.abs(inputs).max()) / 127.0)
    tbl_q = np.clip(np.rint(tbl0 / sg), -127, 127).astype(np.int8)

    nc = _get_nc()

    x_q = np.clip(np.rint(inputs.reshape(B, S * D) / sg), -127, 127).astype(np.int8)
    in_maps = []
    for c in range(N_CORES):
        x_shard = np.ascontiguousarray(
            x_q[c * B_PER : (c + 1) * B_PER].reshape(NTOK, D)
        )
        cat_shard = cat[c * B_PER : (c + 1) * B_PER].reshape(NTOK)
        in_maps.append({"x": x_shard, "tbl": tbl_q, "idx": _prep_idx(cat_shard)})

    res = run_bass_kernel_spmd(
        nc, in_maps, core_ids=list(range(N_CORES)), **RUN_KWARGS
    )
    LAST_RESULTS = res
    out = np.concatenate(
        [
            (r["out"].astype(np.float32) * sg).reshape(B_PER, S, D)
            for r in res.results
        ],
        axis=0,
    )
    return out


# revision 12
# speedup vs baseline: 1.0460x; 1.0460x over previous
"""Trainium2 Bass kernel for nn_CategoryAdder (embedding lookup + masked add).

Computation: out[b,s,:] = inputs[b,s,:] + emb where
  emb = table[categories[b,s]] masked to zero when categories[b,s]==0 or
  s == mask_positions[b].

Host-side preprocessing folds both masks into the data:
  - categories[b, mask_positions[b]] = 0
  - table row 0 zeroed (on a copy)
so the device computes exactly: out = inputs + table0[categories].

Numeric scheme: x and the table are both int8, quantized host-side with ONE
shared scale S = max(absmax(x), absmax(table))/127. The device adds raw int8
codes on the DVE with int8 output — measured on TRN2, the DVE saturates the
int8 writeback to [-128,127], so the rare |qx+qt|>127 sums clip instead of
wrapping (measured rel err 1.28e-2 vs the 2e-2 gate; clipping contributes
~4e-3). The host multiplies by S during the free fp32 conversion. Byte
footprint per core: 8.4 (x) + 8.4 (gather) + 8.4 (out) MB = 25.2MB, a 70us
DMA-bus roofline (16 engines x 22.5 GB/s).

Bottleneck structure (from NTFF profiles): the SWDGE gather's Q7 descriptor
generation runs ~1.9ns/desc uncontended but 6-9ns/desc while the DVE runs
adds (Pool shares an SBUF port with the DVE), and a 2048-desc gather
overflows the default 1024-desc/queue ring so gen stalls mid-instruction on
drain. Mitigations here:
  - Every gather tile is resident in SBUF (qp pool = one buf per tile), so
    gather dispatch never waits on the DVE add chain to recycle a slot.
  - A 16-index warmup gather pays the ~6us first-call Q7 IRAM load while
    the idx DMAs are still in flight.
  - inp bufs=4 so x loads enter the HWDGE ring ahead of the stores
    (a late x load starved the DVE for 10us in one trace).
  - 4 SWDGE queues rotate so drains overlap gen of the next gather.
  - Tile schedule is small at both ends: small head tiles prime the pipe
    (first add by ~8us), small tail tiles shrink the serial
    last-gather -> last-add -> last-store chain.
  - The last tiles' x is preloaded up front so final adds never queue
    behind stores in the HWDGE ring FIFO; adds/stores are pinned to tile
    order with ordering-only deps (the scheduler's cost model
    underestimates gather gen ~20x and otherwise hoists late-tile adds,
    serializing the tail).

Sharding: data-parallel over batch across 8 NeuronCores (8 batches per core,
16384 tokens/core). Table replicated. Per tile of T tokens: SWDGE dma_gather
pulls 512B int8 table rows from HBM by precomputed int16 indices, HWDGE loads
the int8 input tile, DVE adds the codes (saturating int8), HWDGE stores int8.
"""

import numpy as np
import ml_dtypes

import concourse.mybir as mybir
from concourse import bacc, tile
from concourse.bass_utils import run_bass_kernel_spmd
from concourse.tile import add_dep_helper

BF16 = ml_dtypes.bfloat16


def _ensure_axon_ntff_hook_module():
    """run_bass_kernel_spmd(trace=True) under axon imports antenv.axon_hooks,
    which this image lacks — install a fallback shim (backed by the boot
    module's ctypes hook when available) so a BASS_TRACE=1 environment does
    not crash the kernel. No-op when the real module exists."""
    try:
        import antenv.axon_hooks  # noqa: F401
        return
    except ImportError:
        pass
    import sys
    import types

    hook = None
    try:
        import trn_agent_boot.trn_boot as _tb

        hook = _tb._ntff_profile_via_ctypes("/opt/axon/libaxon_pjrt.so")
    except Exception:
        hook = None  # get_..._hook() -> None makes bass_utils skip tracing
    mod = types.ModuleType("antenv.axon_hooks")
    mod.get_axon_ntff_profile_hook = lambda: hook
    mod.set_axon_ntff_profile_hook = lambda h: None
    sys.modules["antenv.axon_hooks"] = mod


_ensure_axon_ntff_hook_module()

B, S, D = 64, 2048, 512
N_CAT = 5000
N_CORES = 8
B_PER = B // N_CORES          # 8 batches per core
NTOK = B_PER * S              # 16384 tokens per core
IDX_COLS = NTOK // 16         # columns of the wrapped int16 index tensor

# Tile schedule (tokens per tile): small tiles prime the pipeline at the start
# (first gather gen is ~0.5us, so the first add can start by ~8us) and small
# tail tiles shorten the serial gather->add->store chain at the end.
TILES = [256, 256, 512, 1024] + [2048] * 6 + [1024, 512, 256, 256]
assert sum(TILES) == NTOK
N_HEAD = 3  # tiles whose indices ride the small head idx DMA
N_TAIL = 3  # tiles whose x is preloaded at start (dedicated pool) so the
            # last adds never wait on x-loads queued behind big stores


def _build_nc():
    nc = bacc.Bacc(
        "TRN2",
        target_bir_lowering=False,
        debug=False,
        num_swdge_queues=4,
        # 2560-desc rings per queue: with every gather dispatched eagerly,
        # all four queues generate concurrently and a 2048-desc gather must
        # fit its ring or gen stalls mid-instruction on drain (measured:
        # default 1024-desc rings cost ~15us in this eager configuration).
        dynamic_dma_scratch_size=40960,
    )
    x = nc.dram_tensor("x", [NTOK, D], mybir.dt.int8, kind="ExternalInput")
    tbl = nc.dram_tensor("tbl", [N_CAT, D], mybir.dt.int8, kind="ExternalInput")
    idx = nc.dram_tensor("idx", [128, IDX_COLS], mybir.dt.int16, kind="ExternalInput")
    out = nc.dram_tensor("out", [NTOK, D], mybir.dt.int8, kind="ExternalOutput")

    head = sum(t // 16 for t in TILES[:N_HEAD])
    with tile.TileContext(nc) as tc:
        with (
            tc.tile_pool(name="idxp", bufs=1) as idxp,
            tc.tile_pool(name="inp", bufs=4) as inp,
            # One resident gather tile per schedule tile: gathers never wait
            # on the DVE add chain to free a slot (v3 trace showed 8.5us
            # per-gather waits on qp reuse), so all SWDGE gen+drain finishes
            # early and the add chain runs gather-unblocked.
            tc.tile_pool(name="qp", bufs=len(TILES)) as qp,
            tc.tile_pool(name="outp", bufs=3) as outp,
            tc.tile_pool(name="tailp", bufs=N_TAIL) as tailp,
        ):
            # Warmup: a 16-index gather of row 0 issued before anything
            # else pays the ~6us first-call Q7 IRAM load and cold-dispatch
            # cost while the idx DMAs are still in flight, so the first real
            # gather starts ~5us earlier.
            warm_idx = idxp.tile([128, 1], mybir.dt.int16, tag="warmi")
            nc.vector.memset(warm_idx[:], 0)
            warm_q = idxp.tile([128, 512], mybir.dt.int8, tag="warmq")
            nc.gpsimd.dma_gather(
                warm_q[:].rearrange("p (c e) -> p c e", e=D),
                tbl[:, :],
                warm_idx[:, :],
                16,
                16,
                D,
                single_packet=False,
                queue_num=0,
            )
            # Two separate idx tiles (separate semaphores): the first gather
            # only waits on the small head DMA, not the full idx transfer.
            idx_head = idxp.tile([128, head], mybir.dt.int16, tag="idxh")
            idx_tail = idxp.tile([128, IDX_COLS - head], mybir.dt.int16, tag="idxt")
            nc.sync.dma_start(out=idx_head[:], in_=idx[:, :head])
            nc.sync.dma_start(out=idx_tail[:], in_=idx[:, head:])
            # Preload the tail tiles' x up front: issued now, these loads sit
            # ahead of all stores in the HWDGE ring FIFO, so the final adds
            # are never stuck behind store drains.
            tail_x = []
            t0 = NTOK - sum(TILES[-N_TAIL:])
            for T in TILES[-N_TAIL:]:
                xt = tailp.tile([128, (T // 128) * D], mybir.dt.int8, tag="tx")
                nc.sync.dma_start(
                    out=xt[:],
                    in_=x[t0 : t0 + T].rearrange("(p c) e -> p (c e)", p=128),
                )
                tail_x.append(xt)
                t0 += T
            t0 = 0
            col = 0
            prev_add = None
            prev_store = None
            for ti, T in enumerate(TILES):
                C = T // 128
                if ti < N_HEAD:
                    idx_ap = idx_head[:, col : col + T // 16]
                else:
                    idx_ap = idx_tail[:, col - head : col - head + T // 16]
                q_t = qp.tile([128, C * D], mybir.dt.int8, tag="q")
                nc.gpsimd.dma_gather(
                    q_t[:].rearrange("p (c e) -> p c e", e=D),
                    tbl[:, :],
                    idx_ap,
                    T,
                    T,
                    D,
                    # multi-packet lets the SDMA engines start draining while
                    # Q7 is still generating descriptors; single_packet also
                    # hard-fails above 1024 idxs.
                    single_packet=False,
                    # Alternate SWDGE queues: separate descriptor rings let
                    # drains overlap the next gather's generation.
                    queue_num=ti % 4,
                )
                if ti >= len(TILES) - N_TAIL:
                    in_t = tail_x[ti - (len(TILES) - N_TAIL)]
                else:
                    in_t = inp.tile([128, C * D], mybir.dt.int8, tag="in")
                    nc.sync.dma_start(
                        out=in_t[:],
                        in_=x[t0 : t0 + T].rearrange("(p c) e -> p (c e)", p=128),
                    )
                # Saturating integer-code add on the DVE: int8+int8 -> int8
                # clamps to [-128,127] on writeback (hardware-verified).
                o_t = outp.tile([128, C * D], mybir.dt.int8, tag="o")
                add_i = nc.vector.tensor_add(out=o_t[:], in0=q_t[:], in1=in_t[:])
                store_i = nc.sync.dma_start(
                    out=out[t0 : t0 + T].rearrange("(p c) e -> p (c e)", p=128),
                    in_=o_t[:],
                )
                # Pin adds/stores to tile order with ordering-only edges:
                # the scheduler's cost model thinks gather gen is ~free and
                # otherwise hoists late-tile adds ahead of mid-tile ones,
                # serializing the tail behind the slowest gathers.
                if prev_add is not None:
                    add_dep_helper(
                        add_i.ins, prev_add.ins, sync=False, reason="pin add order"
                    )
                    add_dep_helper(
                        store_i.ins, prev_store.ins, sync=False,
                        reason="pin store order",
                    )
                prev_add, prev_store = add_i, store_i
                t0 += T
                col += T // 16
    nc.compile()
    return nc


def _prep_idx(cat_shard: np.ndarray) -> np.ndarray:
    """cat_shard: (NTOK,) int -> wrapped int16 index tensor [128, IDX_COLS].

    dma_gather writes gather-slot i to SBUF (partition i%128, column i//128);
    our tiles place token t at (partition t//C, column t%C), so slot i holds
    the category of token (i%128)*C + i//128. Indices are then wrapped 16-way
    (idxs[p, s] = slot s*16+p) and replicated across the 8 groups of 16
    partitions as the HW expects.
    """
    blocks = []
    t0 = 0
    for T in TILES:
        C = T // 128
        slot_to_token = (np.arange(T) % 128) * C + (np.arange(T) // 128)
        vals = cat_shard[t0 : t0 + T][slot_to_token]
        blocks.append(np.tile(vals.reshape(T // 16, 16).T, (8, 1)))
        t0 += T
    return np.ascontiguousarray(np.concatenate(blocks, axis=1).astype(np.int16))


RUN_KWARGS = {}  # test harness can set e.g. {"trace": True}
LAST_RESULTS = None
_NC = None


def _get_nc():
    global _NC
    if _NC is None:
        _NC = _build_nc()
    return _NC


def kernel(inputs, categories, mask_positions, table):
    global LAST_RESULTS
    inputs = np.asarray(inputs, dtype=np.float32)
    categories = np.asarray(categories).astype(np.int64)
    mask_positions = np.asarray(mask_positions).astype(np.int64)
    table = np.asarray(table, dtype=np.float32)

    # Fold both masks into the data.
    cat = categories.copy()
    cat[np.arange(B), mask_positions[:, 0]] = 0
    tbl0 = table.astype(np.float32)
    tbl0[0] = 0.0
    sg = np.float32(max(np.abs(tbl0).max(), np.abs(inputs).max()) / 127.0)
    tbl_q = np.clip(np.rint(tbl0 / sg), -127, 127).astype(np.int8)

    nc = _get_nc()

    x_q = np.clip(np.rint(inputs.reshape(B, S * D) / sg), -127, 127).astype(np.int8)
    in_maps = []
    for c in range(N_CORES):
        x_shard = np.ascontiguousarray(
            x_q[c * B_PER : (c + 1) * B_PER].reshape(NTOK, D)
        )
        cat_shard = cat[c * B_PER : (c + 1) * B_PER].reshape(NTOK)
        in_maps.append({"x": x_shard, "tbl": tbl_q, "idx": _prep_idx(cat_shard)})

    res = run_bass_kernel_spmd(
        nc, in_maps, core_ids=list(range(N_CORES)), **RUN_KWARGS
    )
    LAST_RESULTS = res
    out = np.concatenate(
        [
            (r["out"].astype(np.float32) * sg).reshape(B_PER, S, D)
            for r in res.results
        ],
        axis=0,
    )
    return out
